# revision 4
# baseline (speedup 1.0000x reference)
"""Banded local-linear layer (nn_LocalLinearLayer) on 8 trn2 NeuronCores.

out[b, o, c] = sum_p W[o, p] * xpad[b, c, p] + bias[o],  band p in [o, o+25)
xpad = edge-replicate pad of x along L (first/last 12 rows duplicated).

Strategy:
  - Data-parallel over batch: 4 batches per core; banded weights/bias replicated.
  - Materialize xpad (4120 rows) in SBUF with row r on partition r%128, so the
    128-row output tile t contracts over xpad rows [t*128, t*128+152):
    two PSUM-accumulated matmuls, K=128 (xp tile t) + K=24 (xp tile t+1),
    both at base partition 0. N = 4 batches x 64 channels = 256.
  - Weights host-prepared as transposed banded blocks (lhsT layout).
  - ScalarE activation(Identity, bias) moves PSUM->SBUF and adds bias.
  - x/out staged in 8-tile chunks so compute overlaps the streaming DMAs.
"""

import sys

for _p in ("/opt/trn_rl_repo",):
    if _p not in sys.path:
        sys.path.insert(0, _p)

import numpy as np

import concourse.bass as bass
import concourse.tile as tile
from concourse import bacc, mybir
from concourse.bass_utils import run_bass_kernel_spmd

L = 4096
WIN = 25
PAD = (WIN - 1) // 2  # 12
PADDED = L + 2 * PAD  # 4120
B = 32
C = 64
NCORES = 8
BPC = B // NCORES  # 4 batches per core
P = 128
NT = L // P  # 32 output tiles of 128 rows
KE = WIN - 1  # 24 = extra contraction rows from xp tile t+1
NFREE = BPC * C  # 256 (matmul moving free dim)
NCHUNK = 4
TPC = NT // NCHUNK  # 8 tiles per chunk

F32 = mybir.dt.float32


def _host_weights(W: np.ndarray, b: np.ndarray):
    """Banded W (masked, padded coords) -> transposed per-tile lhsT blocks."""
    o = np.arange(L)[:, None]
    p = np.arange(PADDED)[None, :]
    Wm = np.where((p >= o) & (p < o + WIN), W, 0.0).astype(np.float32)

    # wb[k, t, m]    = Wm[t*128 + m, t*128 + k]        k in [0,128)
    # wedge[k, t, m] = Wm[t*128 + m, t*128 + 128 + k]  k in [0,24)
    wb = np.zeros((P, NT, P), np.float32)
    wedge = np.zeros((KE, NT, P), np.float32)
    for t in range(NT):
        wb[:, t, :] = Wm[t * P : (t + 1) * P, t * P : (t + 1) * P].T
        wedge[:, t, :] = Wm[t * P : (t + 1) * P, (t + 1) * P : (t + 1) * P + KE].T

    bias_t = np.ascontiguousarray(b.reshape(NT, P).T).astype(np.float32)  # [128, NT]
    return wb, wedge, bias_t


def _build_nc():
    nc = bacc.Bacc("TRN2", target_bir_lowering=False, debug=False, num_devices=NCORES)
    x_d = nc.dram_tensor("x", [BPC, L, C], F32, kind="ExternalInput").ap()
    wb_d = nc.dram_tensor("wb", [P, NT, P], F32, kind="ExternalInput").ap()
    we_d = nc.dram_tensor("wedge", [KE, NT, P], F32, kind="ExternalInput").ap()
    bias_d = nc.dram_tensor("bias", [P, NT], F32, kind="ExternalInput").ap()
    out_d = nc.dram_tensor("out", [BPC, L, C], F32, kind="ExternalOutput").ap()

    with tile.TileContext(nc) as tc:
        with (
            tc.tile_pool(name="main", bufs=1) as pool,
            tc.tile_pool(name="ps", bufs=8, space=bass.MemorySpace.PSUM) as pspool,
        ):
            wb_s = pool.tile([P, NT, P], F32)
            we_s = pool.tile([KE, NT, P], F32)
            bias_s = pool.tile([P, NT], F32)
            # xp chunks: chunk c holds xp tiles c*8 .. c*8+7; xtail = xp tile 32
            # (24 rows).  sch = output staging chunks.
            xch = [
                pool.tile([P, TPC, NFREE], F32, name=f"xch{c}") for c in range(NCHUNK)
            ]
            xtail = pool.tile([KE, NFREE], F32)
            sch = [
                pool.tile([P, TPC, NFREE], F32, name=f"sch{c}") for c in range(NCHUNK)
            ]

            nc.sync.dma_start(wb_s[:], wb_d)
            nc.sync.dma_start(we_s[:], we_d)
            nc.sync.dma_start(bias_s[:], bias_d)

            # xp row j*128+p corresponds to x row j*128+p-12, with
            # xp rows 0:12 = x rows 0:12 and xp rows 4108:4120 = x rows 4084:4096.
            for ch in range(NCHUNK):
                for b in range(BPC):
                    dstc = xch[ch][:, :, b * C : (b + 1) * C]
                    if ch == 0:
                        # tile 0: partitions 0:12 <- x rows 0:12 (edge dup),
                        #         partitions 12:128 <- x rows 0:116
                        nc.sync.dma_start(dstc[0:12, 0, :], x_d[b][0:12])
                        nc.sync.dma_start(dstc[12:, 0, :], x_d[b][0:116])
                        # tiles 1..7 in one strided DMA: x rows 116 .. 116+7*128
                        src = x_d[b][116 : 116 + (TPC - 1) * P].rearrange(
                            "(t p) c -> p t c", p=P
                        )
                        nc.sync.dma_start(dstc[:, 1:TPC, :], src)
                    else:
                        j0 = ch * TPC
                        src = x_d[b][j0 * P - PAD : j0 * P - PAD + TPC * P].rearrange(
                            "(t p) c -> p t c", p=P
                        )
                        nc.sync.dma_start(dstc[:, :, :], src)
            for b in range(BPC):
                # xp tile 32: parts 0:12 <- x rows 4084:4096, parts 12:24 same
                nc.sync.dma_start(xtail[0:12, b * C : (b + 1) * C], x_d[b][L - PAD :])
                nc.sync.dma_start(xtail[12:24, b * C : (b + 1) * C], x_d[b][L - PAD :])

            def xptile(j):  # xp tile j as [128 (or 24 for tail), 256]
                return xtail[:] if j == NT else xch[j // TPC][:, j % TPC, :]

            for t in range(NT):
                ps = pspool.tile([P, NFREE], F32)
                nc.tensor.matmul(ps[:], wb_s[:, t], xptile(t), start=True, stop=False)
                nc.tensor.matmul(
                    ps[:], we_s[:, t], xptile(t + 1)[:KE], start=False, stop=True
                )
                nc.scalar.activation(
                    sch[t // TPC][:, t % TPC, :],
                    ps[:],
                    mybir.ActivationFunctionType.Identity,
                    bias=bias_s[:, t : t + 1],
                )

            for ch in range(NCHUNK):
                for b in range(BPC):
                    dst = out_d[b].rearrange("(t p) c -> p t c", p=P)
                    nc.sync.dma_start(
                        dst[:, ch * TPC : (ch + 1) * TPC, :],
                        sch[ch][:, :, b * C : (b + 1) * C],
                    )

    nc.compile()
    return nc


_NC = None


def _get_nc():
    global _NC
    if _NC is None:
        _NC = _build_nc()
    return _NC


def kernel(x: np.ndarray, W: np.ndarray, b: np.ndarray) -> np.ndarray:
    x = np.ascontiguousarray(x, dtype=np.float32)
    wb, wedge, bias_t = _host_weights(
        np.asarray(W, dtype=np.float32), np.asarray(b, dtype=np.float32)
    )
    nc = _get_nc()
    in_maps = [
        {"x": x[c * BPC : (c + 1) * BPC], "wb": wb, "wedge": wedge, "bias": bias_t}
        for c in range(NCORES)
    ]
    res = run_bass_kernel_spmd(nc, in_maps, list(range(NCORES)))
    out = np.concatenate([r["out"] for r in res.results], axis=0)
    return out.astype(np.float32)


if __name__ == "__main__":
    rng = np.random.default_rng(0)
    x = rng.standard_normal((B, L, C), dtype=np.float32)
    W = rng.standard_normal((L, PADDED), dtype=np.float32) * 0.02
    b = rng.standard_normal((L,), dtype=np.float32) * 0.02
    out = kernel(x, W, b)
    print(out.shape, out.dtype)


# revision 8
# speedup vs baseline: 2.3232x; 2.3232x over previous
"""Banded local-linear layer (nn_LocalLinearLayer) on 8 trn2 NeuronCores.

out[b, o, c] = sum_p W[o, p] * xpad[b, c, p] + bias[o],  band p in [o, o+25)
xpad = edge-replicate pad of x along L (first/last 12 rows duplicated).

Strategy (v3):
  - Data-parallel over batch: 4 batches per core; banded weights replicated.
  - Output tiled in 104-row tiles: tile t = out rows [104t, 104t+104), contracts
    over xpad rows [104t, 104t+128) -> ONE K=128 matmul per tile (40 tiles).
  - Host pre-shuffles xpad into the exact SBUF layout [128, tile, b*64+c] (fp16)
    and unshuffles the output, so every DMA is fully contiguous (large
    descriptors, no strided-DMA penalty) and the device loop is uniform.
  - fp16 operands, fp32 PSUM accumulation, fp32 bias/output (~4e-4 rel err).
  - PSUM->SBUF + bias alternates ScalarE activation / VectorE tensor_scalar_add.
  - x/out staged in 4 chunks of 10 tiles for DMA/compute overlap; input DMAs on
    the Sync HWDGE ring, output DMAs on the Scalar ring.
"""

import sys

for _p in ("/opt/trn_rl_repo",):
    if _p not in sys.path:
        sys.path.insert(0, _p)

import numpy as np

import concourse.bass as bass
import concourse.tile as tile
from concourse import bacc, mybir
from concourse.bass_utils import run_bass_kernel_spmd

L = 4096
WIN = 25
PAD = (WIN - 1) // 2  # 12
PADDED = L + 2 * PAD  # 4120
B = 32
C = 64
NCORES = 8
BPC = B // NCORES  # 4
P = 128
M = P - (WIN - 1)  # 104 output rows per tile
NT = (L + M - 1) // M  # 40 tiles
M_LAST = L - (NT - 1) * M  # 40
NFREE = BPC * C  # 256
NCHUNK = 4
TPC = NT // NCHUNK  # 10

F32 = mybir.dt.float32
F16 = mybir.dt.float16


def _host_weights(W: np.ndarray, b: np.ndarray):
    o = np.arange(L)[:, None]
    p = np.arange(PADDED)[None, :]
    Wm = np.where((p >= o) & (p < o + WIN), W, 0.0).astype(np.float32)
    # wb[k, t, m] = Wm[t*104+m, t*104+k], zero-padded out of range
    wb = np.zeros((P, NT, M), np.float32)
    bias_t = np.zeros((M, NT), np.float32)
    for t in range(NT):
        mt = min(M, L - t * M)
        kt = min(P, PADDED - t * M)
        wb[:kt, t, :mt] = Wm[t * M : t * M + mt, t * M : t * M + kt].T
        bias_t[:mt, t] = b[t * M : t * M + mt]
    return wb.astype(np.float16), bias_t


def _host_x(x: np.ndarray):
    """x [B, L, C] f32 -> [P, NT, B, C] f16 in xpad-tile layout."""
    xp = np.concatenate([x[:, :PAD], x, x[:, -PAD:]], axis=1).astype(np.float16)
    xh = np.zeros((P, NT, B, C), np.float16)
    for t in range(NT):
        kt = min(P, PADDED - t * M)
        xh[:kt, t] = xp[:, t * M : t * M + kt].transpose(1, 0, 2)
    return xh


def _build_nc():
    nc = bacc.Bacc("TRN2", target_bir_lowering=False, debug=False, num_devices=NCORES)
    x_d = nc.dram_tensor("x", [P, NT, NFREE], F16, kind="ExternalInput").ap()
    wb_d = nc.dram_tensor("wb", [P, NT, M], F16, kind="ExternalInput").ap()
    bias_d = nc.dram_tensor("bias", [M, NT], F32, kind="ExternalInput").ap()
    out_d = nc.dram_tensor("out", [M, NT, NFREE], F32, kind="ExternalOutput").ap()

    with tile.TileContext(nc) as tc:
        with (
            tc.tile_pool(name="main", bufs=1) as pool,
            tc.tile_pool(name="ps", bufs=8, space=bass.MemorySpace.PSUM) as pspool,
        ):
            wb_s = pool.tile([P, NT, M], F16)
            bias_s = pool.tile([M, NT], F32)
            xch = [
                pool.tile([P, TPC, NFREE], F16, name=f"xch{c}") for c in range(NCHUNK)
            ]
            sch = [
                pool.tile([M, TPC, NFREE], F32, name=f"sch{c}") for c in range(NCHUNK)
            ]

            nc.sync.dma_start(wb_s[:], wb_d)
            nc.sync.dma_start(bias_s[:], bias_d)
            for ch in range(NCHUNK):
                nc.sync.dma_start(
                    xch[ch][:], x_d[:, ch * TPC : (ch + 1) * TPC, :]
                )

            for t in range(NT):
                c, j = t // TPC, t % TPC
                ps = pspool.tile([M, NFREE], F32)
                nc.tensor.matmul(
                    ps[:], wb_s[:, t], xch[c][:, j, :], start=True, stop=True
                )
                if t % 2 == 0:
                    nc.scalar.activation(
                        sch[c][:, j, :],
                        ps[:],
                        mybir.ActivationFunctionType.Identity,
                        bias=bias_s[:, t : t + 1],
                    )
                else:
                    nc.vector.tensor_scalar_add(
                        sch[c][:, j, :], ps[:], bias_s[:, t : t + 1]
                    )

            for ch in range(NCHUNK):
                nc.scalar.dma_start(
                    out_d[:, ch * TPC : (ch + 1) * TPC, :], sch[ch][:]
                )

    nc.compile()
    return nc


_NC = None


def _get_nc():
    global _NC
    if _NC is None:
        _NC = _build_nc()
    return _NC


def _make_in_maps(x, W, b):
    wb, bias_t = _host_weights(
        np.asarray(W, dtype=np.float32), np.asarray(b, dtype=np.float32)
    )
    xh = _host_x(np.asarray(x, dtype=np.float32))
    return [
        {
            "x": np.ascontiguousarray(
                xh[:, :, c * BPC : (c + 1) * BPC, :]
            ).reshape(P, NT, NFREE),
            "wb": wb,
            "bias": bias_t,
        }
        for c in range(NCORES)
    ]


def _gather(results):
    oh = np.concatenate(
        [r["out"].reshape(M, NT, BPC, C) for r in results], axis=2
    )  # [104, NT, B, C]
    out = np.empty((B, L, C), np.float32)
    for t in range(NT):
        mt = min(M, L - t * M)
        out[:, t * M : t * M + mt] = oh[:mt, t].transpose(1, 0, 2)
    return out


def kernel(x: np.ndarray, W: np.ndarray, b: np.ndarray) -> np.ndarray:
    nc = _get_nc()
    res = run_bass_kernel_spmd(nc, _make_in_maps(x, W, b), list(range(NCORES)))
    return _gather(res.results)


if __name__ == "__main__":
    rng = np.random.default_rng(0)
    x = rng.standard_normal((B, L, C), dtype=np.float32)
    W = rng.standard_normal((L, PADDED), dtype=np.float32) * 0.02
    b = rng.standard_normal((L,), dtype=np.float32) * 0.02
    print(kernel(x, W, b).shape)
